# revision 14
# baseline (speedup 1.0000x reference)
"""GQA causal attention kernel for 8 Trainium2 NeuronCores.

Sharding: data-parallel over batch (2) x tensor-parallel over head groups (4).
Core c handles batch b = c // 4 and head group g = c % 4 (query heads
4g..4g+3, KV head g, Wo rows 512g..512(g+1)).  Each core computes a full
[N, DIM] partial of the output projection; the host sums the 4 partials
per batch.

Matmuls run in bf16 (fp32r measured ~2 cycles/row on HW, bf16 1): inputs are
converted on the host; all PSUM accumulation stays fp32.

Per-core pipeline:
  1. QKV projections from host-pretransposed x^T; all 16 D-chunks resident in
     SBUF (bf16), full-depth PSUM accumulation, 3 waves of 8 PSUM banks.
  2. V^T -> V via PE transposes.
  3. Per (head, 512-wide q band): scores computed TRANSPOSED (S^T[k, q]) so
     softmax needs no P transposes; exp on ScalarE; softmax denominators
     accumulated in broadcast form via an all-ones [128,128] stationary
     matmul (one PSUM bank holds 128 identical rows of the row sums, so
     normalization is reciprocal + one multiply, all 128-partition DVE ops);
     O^T accumulated over k chunks in PSUM.
  4. Output projection consumes O^T directly as the stationary operand.
"""

import os
import numpy as np

B, N, DIM = 2, 2048, 2048
H, KVH, HD = 16, 4, 128
HQ = H // KVH          # query heads per core
SCALE = float(HD) ** -0.5
NT = N // 128          # 16 seq tiles
DC = DIM // 128        # 16 contraction chunks
NB = 4                 # q bands of 512
BW = N // NB           # 512 band width

_cache = {}


def _build():
    import concourse.bass as bass
    import concourse.bacc as bacc
    import concourse.tile as tile
    import concourse.mybir as mybir

    f32 = mybir.dt.float32
    bf16 = mybir.dt.bfloat16
    EXP = mybir.ActivationFunctionType.Exp

    nc = bacc.Bacc("TRN2", target_bir_lowering=False, debug=False)

    xT = nc.dram_tensor("xT", [DIM, N], bf16, kind="ExternalInput")
    wq = nc.dram_tensor("wq", [DIM, HQ * HD], bf16, kind="ExternalInput")
    wk = nc.dram_tensor("wk", [DIM, HD], bf16, kind="ExternalInput")
    wv = nc.dram_tensor("wv", [DIM, HD], bf16, kind="ExternalInput")
    wo = nc.dram_tensor("wo", [HQ * HD, DIM], bf16, kind="ExternalInput")
    m01 = nc.dram_tensor("m01", [128, 4 * BW], bf16, kind="ExternalInput")
    ident = nc.dram_tensor("ident", [128, 128], bf16, kind="ExternalInput")
    onesd = nc.dram_tensor("onesd", [128, 128], bf16, kind="ExternalInput")
    out = nc.dram_tensor("out", [N, DIM], f32, kind="ExternalOutput")

    with tile.TileContext(nc) as tc:
        from contextlib import ExitStack

        with ExitStack() as ctx:
            resident = ctx.enter_context(tc.tile_pool(name="resident", bufs=1))

            # --- resident tiles ---
            qt = resident.tile([128, HQ * N], bf16)        # Q^T all heads
            kt = resident.tile([128, N], bf16)             # K^T
            vnat = resident.tile([128, N], bf16)           # V (seq-major chunks)
            m01_sb = resident.tile([128, 4 * BW], bf16)
            id_sb = resident.tile([128, 128], bf16)
            ones_sb = resident.tile([128, 128], bf16)
            wo_sb = [resident.tile([128, DIM], bf16, tag=f"wo{h}", name=f"wo{h}")
                     for h in range(HQ)]
            ot_sb2 = [resident.tile([128, HQ * BW], bf16, tag=f"ot_sb{i}",
                                    name=f"ot_sb{i}") for i in range(2)]

            # ---------------- Phase 1: projections ----------------
            with nc.named_scope("proj"):
                with (
                    tc.tile_pool(name="xth", bufs=1) as xth_pool,
                    tc.tile_pool(name="wqh", bufs=1) as wqh_pool,
                    tc.tile_pool(name="wkv", bufs=1) as wkv_pool,
                    tc.tile_pool(name="pp", bufs=8, space="PSUM") as pp,
                    tc.tile_pool(name="vt", bufs=1) as vt_pool,
                ):
                    # PE warmup while input DMAs land: ~4.5us of matmul
                    # activity flips the HAM clock gate to 8/8 before real
                    # work starts. Reads uninitialized SBUF (values unused);
                    # sink DMA keeps the chain from being dead-code-eliminated.
                    warm = pp.tile([128, 512], f32, tag="acc")
                    for _ in range(20):
                        nc.tensor.matmul(warm[:], ones_sb[:],
                                         m01_sb[:, 0:512])
                    wsink = vt_pool.tile([128, 512], f32, tag="wsink")
                    nc.vector.tensor_copy(wsink[:], warm[:])
                    sink_dram = nc.dram_tensor("warm_sink", [128, 512], f32,
                                               kind="Internal")
                    nc.sync.dma_start(sink_dram.ap(), wsink[:])
                    wk_sb = wkv_pool.tile([128, DC * HD], bf16, tag="wk")
                    wv_sb = wkv_pool.tile([128, DC * HD], bf16, tag="wv")
                    vtmp = vt_pool.tile([128, N], bf16)    # V^T before transpose

                    # x^T / Wq chunks first (first matmuls need them); each
                    # x^T chunk split in two so transfers spread across DMA
                    # queues and matmuls start on the first half.
                    xth = []
                    wqh = []
                    for d in range(DC):
                        wq_t = wqh_pool.tile([128, HQ * HD], bf16, tag=f"w{d}",
                                             name=f"wqh{d}")
                        nc.sync.dma_start(
                            wq_t[:], wq.ap()[d * 128:(d + 1) * 128, :])
                        wqh.append(wq_t)
                        xt_t = xth_pool.tile([128, N], bf16, tag=f"x{d}",
                                             name=f"xth{d}")
                        for hh in range(2):
                            nc.sync.dma_start(
                                xt_t[:, hh * 1024:(hh + 1) * 1024],
                                xT.ap()[d * 128:(d + 1) * 128,
                                        hh * 1024:(hh + 1) * 1024])
                        xth.append(xt_t)

                    # K/V weights as single strided DMAs (needed at wave 2)
                    nc.sync.dma_start(
                        wk_sb[:].rearrange("p (d c) -> p d c", d=DC),
                        wk.ap().rearrange("(d p) c -> p d c", p=128))
                    nc.sync.dma_start(
                        wv_sb[:].rearrange("p (d c) -> p d c", d=DC),
                        wv.ap().rearrange("(d p) c -> p d c", p=128))
                    nc.sync.dma_start(id_sb[:], ident.ap())
                    nc.sync.dma_start(ones_sb[:], onesd.ap())
                    nc.sync.dma_start(m01_sb[:], m01.ap())

                    # waves 0/1: Q^T for head pairs; wave 2: K^T + V^T
                    for wave in range(3):
                        for s in range(8):
                            acc = pp.tile([128, 512], f32, tag="acc")
                            if wave < 2:
                                h = wave * 2 + s // 4
                                t = s % 4
                                for d in range(DC):
                                    nc.tensor.matmul(
                                        acc[:],
                                        wqh[d][:, h * HD:(h + 1) * HD],
                                        xth[d][:, t * 512:(t + 1) * 512],
                                        start=(d == 0), stop=(d == DC - 1))
                                dst = qt[:, h * N + t * 512: h * N + (t + 1) * 512]
                            else:
                                w_sb = wk_sb if s < 4 else wv_sb
                                t = s % 4
                                for d in range(DC):
                                    nc.tensor.matmul(
                                        acc[:],
                                        w_sb[:, d * HD:(d + 1) * HD],
                                        xth[d][:, t * 512:(t + 1) * 512],
                                        start=(d == 0), stop=(d == DC - 1))
                                src_t = kt if s < 4 else vtmp
                                dst = src_t[:, t * 512:(t + 1) * 512]
                            nc.vector.tensor_copy(dst, acc[:])

                    # V^T -> V natural via PE transpose
                    for j in range(NT):
                        tp = pp.tile([128, 128], bf16, tag="acc")
                        nc.tensor.transpose(
                            tp[:], vtmp[:, j * 128:(j + 1) * 128], id_sb[:])
                        nc.vector.tensor_copy(vnat[:, j * 128:(j + 1) * 128], tp[:])

            # wo loads (needed from first outproj; emitted after proj DMAs)
            for h in range(HQ):
                nc.sync.dma_start(wo_sb[h][:], wo.ap()[h * 128:(h + 1) * 128, :])

            # ---------------- Phase 2: attention + out-projection ----------------
            with nc.named_scope("attn"):
                with (
                    tc.tile_pool(name="pt", bufs=6) as pt_pool,
                    tc.tile_pool(name="rr", bufs=2) as rr_pool,
                    tc.tile_pool(name="stage", bufs=4) as stage_pool,
                    tc.tile_pool(name="st", bufs=2, space="PSUM") as st_pool,
                    tc.tile_pool(name="sums", bufs=2, space="PSUM") as sums_pool,
                    tc.tile_pool(name="ot", bufs=2, space="PSUM") as ot_pool,
                ):
                    for I in range(NB):
                        jmax = 4 * I + 3
                        ot_sb = ot_sb2[I % 2]
                        for h in range(HQ):
                            otp = ot_pool.tile([128, BW], f32, tag="ot")
                            smp = sums_pool.tile([128, BW], f32, tag="sums")
                            for p in range((jmax + 1) // 2):
                                stp = st_pool.tile([128, 2 * BW], f32, tag="st")
                                for u in range(2):
                                    j = 2 * p + u
                                    o = j - 4 * I
                                    qlo = max(0, o) * 128
                                    nc.tensor.matmul(
                                        stp[:, u * BW + qlo:(u + 1) * BW],
                                        kt[:, j * 128:(j + 1) * 128],
                                        qt[:, h * N + I * BW + qlo:
                                           h * N + (I + 1) * BW])
                                ptp = pt_pool.tile([128, 2 * BW], bf16, tag="pt")
                                for u in range(2):
                                    j = 2 * p + u
                                    o = j - 4 * I
                                    qlo = max(0, o) * 128
                                    nc.scalar.activation(
                                        ptp[:, u * BW + qlo:(u + 1) * BW],
                                        stp[:, u * BW + qlo:(u + 1) * BW],
                                        EXP, scale=SCALE)
                                    if o >= 0:
                                        # triangular boundary within first 128
                                        # cols of the processed range
                                        nc.vector.tensor_mul(
                                            ptp[:, u * BW + qlo: u * BW + qlo + 128],
                                            ptp[:, u * BW + qlo: u * BW + qlo + 128],
                                            m01_sb[:, 0:128])
                                    pslice = ptp[:, u * BW + qlo:(u + 1) * BW]
                                    # row sums in broadcast form (all-ones lhsT)
                                    nc.tensor.matmul(
                                        smp[:, qlo:], ones_sb[:], pslice,
                                        start=(j == 0), stop=(j == jmax))
                                    nc.tensor.matmul(
                                        otp[:, qlo:], vnat[:, j * 128:(j + 1) * 128],
                                        pslice,
                                        start=(j == 0), stop=(j == jmax))
                            # normalize: O^T * (1/sums), all [128, BW] DVE ops
                            rb_sb = rr_pool.tile([128, BW], f32, tag="rb")
                            nc.vector.reciprocal_approx_fast(rb_sb[:], smp[:])
                            nc.vector.tensor_mul(
                                ot_sb[:, h * BW:(h + 1) * BW], otp[:], rb_sb[:])
                        # out projection for this band
                        for t in range(4):
                            stg = stage_pool.tile([128, DIM], f32, tag="stg")
                            for dt in range(4):
                                opp = ot_pool.tile([128, 512], f32, tag="ot")
                                for h in range(HQ):
                                    nc.tensor.matmul(
                                        opp[:],
                                        ot_sb[:, h * BW + t * 128: h * BW + (t + 1) * 128],
                                        wo_sb[h][:, dt * 512:(dt + 1) * 512],
                                        start=(h == 0), stop=(h == HQ - 1))
                                nc.vector.tensor_copy(
                                    stg[:, dt * 512:(dt + 1) * 512], opp[:])
                            nc.sync.dma_start(
                                out.ap()[I * BW + t * 128: I * BW + (t + 1) * 128, :],
                                stg[:])

    nc.compile()
    return nc


def _get_nc():
    if "nc" not in _cache:
        _cache["nc"] = _build()
    return _cache["nc"]


def _host_inputs(x, Wq, Wk, Wv, Wo):
    import ml_dtypes
    bf = ml_dtypes.bfloat16
    x = np.asarray(x, dtype=np.float32)
    Wq = np.asarray(Wq, dtype=bf)
    Wk = np.asarray(Wk, dtype=bf)
    Wv = np.asarray(Wv, dtype=bf)
    Wo = np.asarray(Wo, dtype=bf)

    m01 = np.zeros((128, 4 * BW), dtype=bf)
    for o in range(4):
        kk = np.arange(128)[:, None]
        qq = np.arange(BW)[None, :]
        m01[:, o * BW:(o + 1) * BW] = (qq >= o * 128 + kk).astype(bf)
    ident = np.eye(128, dtype=bf)
    onesd = np.ones((128, 128), dtype=bf)

    xTb = [np.ascontiguousarray(x[b].T).astype(bf) for b in range(B)]
    in_maps = []
    for c in range(8):
        b, g = c // 4, c % 4
        in_maps.append({
            "xT": xTb[b],
            "wq": np.ascontiguousarray(Wq[:, g * 512:(g + 1) * 512]),
            "wk": np.ascontiguousarray(Wk[:, g * HD:(g + 1) * HD]),
            "wv": np.ascontiguousarray(Wv[:, g * HD:(g + 1) * HD]),
            "wo": np.ascontiguousarray(Wo[g * 512:(g + 1) * 512, :]),
            "m01": m01,
            "ident": ident,
            "onesd": onesd,
        })
    return in_maps


def run(x, mask, Wq, Wk, Wv, Wo, trace=False, trace_cores=None):
    from concourse.bass_utils import run_bass_kernel_spmd

    nc = _get_nc()
    in_maps = _host_inputs(x, Wq, Wk, Wv, Wo)
    res = run_bass_kernel_spmd(
        nc, in_maps, core_ids=list(range(8)), trace=trace,
        trace_cores=trace_cores)
    full = np.empty((B, N, DIM), dtype=np.float32)
    for b in range(B):
        acc = res.results[b * 4 + 0]["out"].astype(np.float32).copy()
        for g in range(1, 4):
            acc += res.results[b * 4 + g]["out"]
        full[b] = acc
    return full, res


def kernel(x, mask, Wq, Wk, Wv, Wo):
    out, _ = run(x, mask, Wq, Wk, Wv, Wo, trace=False)
    return out


# revision 15
# speedup vs baseline: 1.0107x; 1.0107x over previous
"""GQA causal attention kernel for 8 Trainium2 NeuronCores.

Sharding: data-parallel over batch (2) x tensor-parallel over head groups (4).
Core c handles batch b = c // 4 and head group g = c % 4 (query heads
4g..4g+3, KV head g, Wo rows 512g..512(g+1)).  Each core computes a full
[N, DIM] partial of the output projection; the host sums the 4 partials
per batch.

Matmuls run in bf16 (fp32r measured ~2 cycles/row on HW, bf16 1): inputs are
converted on the host; all PSUM accumulation stays fp32.

Per-core pipeline:
  1. QKV projections from host-pretransposed x^T; all 16 D-chunks resident in
     SBUF (bf16), full-depth PSUM accumulation, 3 waves of 8 PSUM banks.
  2. V^T -> V via PE transposes.
  3. Per (head, 512-wide q band): scores computed TRANSPOSED (S^T[k, q]) so
     softmax needs no P transposes; exp on ScalarE; softmax denominators
     accumulated in broadcast form via an all-ones [128,128] stationary
     matmul (one PSUM bank holds 128 identical rows of the row sums, so
     normalization is reciprocal + one multiply, all 128-partition DVE ops);
     O^T accumulated over k chunks in PSUM.
  4. Output projection consumes O^T directly as the stationary operand.
"""

import os
import numpy as np

B, N, DIM = 2, 2048, 2048
H, KVH, HD = 16, 4, 128
HQ = H // KVH          # query heads per core
SCALE = float(HD) ** -0.5
NT = N // 128          # 16 seq tiles
DC = DIM // 128        # 16 contraction chunks
NB = 4                 # q bands of 512
BW = N // NB           # 512 band width

_cache = {}


def _build():
    import concourse.bass as bass
    import concourse.bacc as bacc
    import concourse.tile as tile
    import concourse.mybir as mybir

    f32 = mybir.dt.float32
    bf16 = mybir.dt.bfloat16
    EXP = mybir.ActivationFunctionType.Exp

    nc = bacc.Bacc("TRN2", target_bir_lowering=False, debug=False)

    xT = nc.dram_tensor("xT", [DIM, N], bf16, kind="ExternalInput")
    wq = nc.dram_tensor("wq", [DIM, HQ * HD], bf16, kind="ExternalInput")
    wk = nc.dram_tensor("wk", [DIM, HD], bf16, kind="ExternalInput")
    wv = nc.dram_tensor("wv", [DIM, HD], bf16, kind="ExternalInput")
    wo = nc.dram_tensor("wo", [HQ * HD, DIM], bf16, kind="ExternalInput")
    m01 = nc.dram_tensor("m01", [128, 4 * BW], bf16, kind="ExternalInput")
    ident = nc.dram_tensor("ident", [128, 128], bf16, kind="ExternalInput")
    onesd = nc.dram_tensor("onesd", [128, 128], bf16, kind="ExternalInput")
    out = nc.dram_tensor("out", [N, DIM], f32, kind="ExternalOutput")

    with tile.TileContext(nc) as tc:
        from contextlib import ExitStack

        with ExitStack() as ctx:
            resident = ctx.enter_context(tc.tile_pool(name="resident", bufs=1))

            # --- resident tiles ---
            qt = resident.tile([128, HQ * N], bf16)        # Q^T all heads
            kt = resident.tile([128, N], bf16)             # K^T
            vnat = resident.tile([128, N], bf16)           # V (seq-major chunks)
            m01_sb = resident.tile([128, 4 * BW], bf16)
            id_sb = resident.tile([128, 128], bf16)
            ones_sb = resident.tile([128, 128], bf16)
            wo_sb = [resident.tile([128, DIM], bf16, tag=f"wo{h}", name=f"wo{h}")
                     for h in range(HQ)]
            ot_sb2 = [resident.tile([128, HQ * BW], bf16, tag=f"ot_sb{i}",
                                    name=f"ot_sb{i}") for i in range(2)]

            # ---------------- Phase 1: projections ----------------
            with nc.named_scope("proj"):
                with (
                    tc.tile_pool(name="xth", bufs=1) as xth_pool,
                    tc.tile_pool(name="wqh", bufs=1) as wqh_pool,
                    tc.tile_pool(name="wkv", bufs=1) as wkv_pool,
                    tc.tile_pool(name="pp", bufs=8, space="PSUM") as pp,
                    tc.tile_pool(name="vt", bufs=1) as vt_pool,
                ):
                    # PE warmup while input DMAs land: ~4.5us of matmul
                    # activity flips the HAM clock gate to 8/8 before real
                    # work starts. Reads uninitialized SBUF (values unused);
                    # sink DMA keeps the chain from being dead-code-eliminated.
                    warm = pp.tile([128, 512], f32, tag="acc")
                    for _ in range(20):
                        nc.tensor.matmul(warm[:], ones_sb[:],
                                         m01_sb[:, 0:512])
                    wsink = vt_pool.tile([128, 512], f32, tag="wsink")
                    nc.vector.tensor_copy(wsink[:], warm[:])
                    sink_dram = nc.dram_tensor("warm_sink", [128, 512], f32,
                                               kind="Internal")
                    nc.sync.dma_start(sink_dram.ap(), wsink[:])
                    wk_sb = wkv_pool.tile([128, DC * HD], bf16, tag="wk")
                    wv_sb = wkv_pool.tile([128, DC * HD], bf16, tag="wv")
                    vtmp = vt_pool.tile([128, N], bf16)    # V^T before transpose

                    # x^T / Wq chunks first (first matmuls need them); each
                    # x^T chunk split in two so transfers spread across DMA
                    # queues and matmuls start on the first half.
                    xth = []
                    wqh = []
                    for d in range(DC):
                        wq_t = wqh_pool.tile([128, HQ * HD], bf16, tag=f"w{d}",
                                             name=f"wqh{d}")
                        nc.sync.dma_start(
                            wq_t[:], wq.ap()[d * 128:(d + 1) * 128, :])
                        wqh.append(wq_t)
                        xt_t = xth_pool.tile([128, N], bf16, tag=f"x{d}",
                                             name=f"xth{d}")
                        for hh in range(2):
                            nc.sync.dma_start(
                                xt_t[:, hh * 1024:(hh + 1) * 1024],
                                xT.ap()[d * 128:(d + 1) * 128,
                                        hh * 1024:(hh + 1) * 1024])
                        xth.append(xt_t)

                    # K/V weights as single strided DMAs (needed at wave 2)
                    nc.sync.dma_start(
                        wk_sb[:].rearrange("p (d c) -> p d c", d=DC),
                        wk.ap().rearrange("(d p) c -> p d c", p=128))
                    nc.sync.dma_start(
                        wv_sb[:].rearrange("p (d c) -> p d c", d=DC),
                        wv.ap().rearrange("(d p) c -> p d c", p=128))
                    nc.sync.dma_start(id_sb[:], ident.ap())
                    nc.sync.dma_start(ones_sb[:], onesd.ap())
                    nc.sync.dma_start(m01_sb[:], m01.ap())

                    # waves 0/1: Q^T for head pairs; wave 2: K^T + V^T
                    for wave in range(3):
                        for s in range(8):
                            acc = pp.tile([128, 512], f32, tag="acc")
                            if wave < 2:
                                h = wave * 2 + s // 4
                                t = s % 4
                                for d in range(DC):
                                    nc.tensor.matmul(
                                        acc[:],
                                        wqh[d][:, h * HD:(h + 1) * HD],
                                        xth[d][:, t * 512:(t + 1) * 512],
                                        start=(d == 0), stop=(d == DC - 1))
                                dst = qt[:, h * N + t * 512: h * N + (t + 1) * 512]
                            else:
                                w_sb = wk_sb if s < 4 else wv_sb
                                t = s % 4
                                for d in range(DC):
                                    nc.tensor.matmul(
                                        acc[:],
                                        w_sb[:, d * HD:(d + 1) * HD],
                                        xth[d][:, t * 512:(t + 1) * 512],
                                        start=(d == 0), stop=(d == DC - 1))
                                src_t = kt if s < 4 else vtmp
                                dst = src_t[:, t * 512:(t + 1) * 512]
                            nc.vector.tensor_copy(dst, acc[:])

                    # V^T -> V natural via PE transpose
                    for j in range(NT):
                        tp = pp.tile([128, 128], bf16, tag="acc")
                        nc.tensor.transpose(
                            tp[:], vtmp[:, j * 128:(j + 1) * 128], id_sb[:])
                        nc.vector.tensor_copy(vnat[:, j * 128:(j + 1) * 128], tp[:])

            # wo loads (needed from first outproj; emitted after proj DMAs)
            for h in range(HQ):
                nc.sync.dma_start(wo_sb[h][:], wo.ap()[h * 128:(h + 1) * 128, :])

            # ---------------- Phase 2: attention + out-projection ----------------
            with nc.named_scope("attn"):
                with (
                    tc.tile_pool(name="pt", bufs=4) as pt_pool,
                    tc.tile_pool(name="rr", bufs=2) as rr_pool,
                    tc.tile_pool(name="stage", bufs=4) as stage_pool,
                    tc.tile_pool(name="st", bufs=2, space="PSUM") as st_pool,
                    tc.tile_pool(name="sums", bufs=2, space="PSUM") as sums_pool,
                    tc.tile_pool(name="ot", bufs=2, space="PSUM") as ot_pool,
                ):
                    for I in range(NB):
                        jmax = 4 * I + 3
                        ot_sb = ot_sb2[I % 2]
                        for h in range(HQ):
                            otp = ot_pool.tile([128, BW], f32, tag="ot")
                            smp = sums_pool.tile([128, BW], f32, tag="sums")
                            for p in range((jmax + 1) // 2):
                                stp = st_pool.tile([128, 2 * BW], f32, tag="st")
                                for u in range(2):
                                    j = 2 * p + u
                                    o = j - 4 * I
                                    qlo = max(0, o) * 128
                                    nc.tensor.matmul(
                                        stp[:, u * BW + qlo:(u + 1) * BW],
                                        kt[:, j * 128:(j + 1) * 128],
                                        qt[:, h * N + I * BW + qlo:
                                           h * N + (I + 1) * BW])
                                ptp = pt_pool.tile([128, 2 * BW], bf16, tag="pt")
                                nc.scalar.activation(ptp[:], stp[:], EXP, scale=SCALE)
                                for u in range(2):
                                    j = 2 * p + u
                                    o = j - 4 * I
                                    qlo = max(0, o) * 128
                                    if o >= 0:
                                        # triangular boundary within first 128
                                        # cols of the processed range
                                        nc.vector.tensor_mul(
                                            ptp[:, u * BW + qlo: u * BW + qlo + 128],
                                            ptp[:, u * BW + qlo: u * BW + qlo + 128],
                                            m01_sb[:, 0:128])
                                    pslice = ptp[:, u * BW + qlo:(u + 1) * BW]
                                    # row sums in broadcast form (all-ones lhsT)
                                    nc.tensor.matmul(
                                        smp[:, qlo:], ones_sb[:], pslice,
                                        start=(j == 0), stop=(j == jmax))
                                    nc.tensor.matmul(
                                        otp[:, qlo:], vnat[:, j * 128:(j + 1) * 128],
                                        pslice,
                                        start=(j == 0), stop=(j == jmax))
                            # normalize: O^T * (1/sums), all [128, BW] DVE ops
                            rb_sb = rr_pool.tile([128, BW], f32, tag="rb")
                            nc.vector.reciprocal_approx_fast(rb_sb[:], smp[:])
                            nc.vector.tensor_mul(
                                ot_sb[:, h * BW:(h + 1) * BW], otp[:], rb_sb[:])
                        # out projection for this band
                        for t in range(4):
                            stg = stage_pool.tile([128, DIM], f32, tag="stg")
                            for dt in range(4):
                                opp = ot_pool.tile([128, 512], f32, tag="ot")
                                for h in range(HQ):
                                    nc.tensor.matmul(
                                        opp[:],
                                        ot_sb[:, h * BW + t * 128: h * BW + (t + 1) * 128],
                                        wo_sb[h][:, dt * 512:(dt + 1) * 512],
                                        start=(h == 0), stop=(h == HQ - 1))
                                nc.vector.tensor_copy(
                                    stg[:, dt * 512:(dt + 1) * 512], opp[:])
                            nc.sync.dma_start(
                                out.ap()[I * BW + t * 128: I * BW + (t + 1) * 128, :],
                                stg[:])

    nc.compile()
    return nc


def _get_nc():
    if "nc" not in _cache:
        _cache["nc"] = _build()
    return _cache["nc"]


def _host_inputs(x, Wq, Wk, Wv, Wo):
    import ml_dtypes
    bf = ml_dtypes.bfloat16
    x = np.asarray(x, dtype=np.float32)
    Wq = np.asarray(Wq, dtype=bf)
    Wk = np.asarray(Wk, dtype=bf)
    Wv = np.asarray(Wv, dtype=bf)
    Wo = np.asarray(Wo, dtype=bf)

    m01 = np.zeros((128, 4 * BW), dtype=bf)
    for o in range(4):
        kk = np.arange(128)[:, None]
        qq = np.arange(BW)[None, :]
        m01[:, o * BW:(o + 1) * BW] = (qq >= o * 128 + kk).astype(bf)
    ident = np.eye(128, dtype=bf)
    onesd = np.ones((128, 128), dtype=bf)

    xTb = [np.ascontiguousarray(x[b].T).astype(bf) for b in range(B)]
    in_maps = []
    for c in range(8):
        b, g = c // 4, c % 4
        in_maps.append({
            "xT": xTb[b],
            "wq": np.ascontiguousarray(Wq[:, g * 512:(g + 1) * 512]),
            "wk": np.ascontiguousarray(Wk[:, g * HD:(g + 1) * HD]),
            "wv": np.ascontiguousarray(Wv[:, g * HD:(g + 1) * HD]),
            "wo": np.ascontiguousarray(Wo[g * 512:(g + 1) * 512, :]),
            "m01": m01,
            "ident": ident,
            "onesd": onesd,
        })
    return in_maps


def run(x, mask, Wq, Wk, Wv, Wo, trace=False, trace_cores=None):
    from concourse.bass_utils import run_bass_kernel_spmd

    nc = _get_nc()
    in_maps = _host_inputs(x, Wq, Wk, Wv, Wo)
    res = run_bass_kernel_spmd(
        nc, in_maps, core_ids=list(range(8)), trace=trace,
        trace_cores=trace_cores)
    full = np.empty((B, N, DIM), dtype=np.float32)
    for b in range(B):
        acc = res.results[b * 4 + 0]["out"].astype(np.float32).copy()
        for g in range(1, 4):
            acc += res.results[b * 4 + g]["out"]
        full[b] = acc
    return full, res


def kernel(x, mask, Wq, Wk, Wv, Wo):
    out, _ = run(x, mask, Wq, Wk, Wv, Wo, trace=False)
    return out
